# revision 3
# baseline (speedup 1.0000x reference)
"""Data-parallel 3x3 conv (implicit GEMM) for Trainium2, 8 NeuronCores.

Problem: x (32,128,56,56) f32, W (256,1152) f32 [C_out, C_in*KH*KW, taps in
(ci,kh,kw) order], b (256,), stride 1, pad 1 -> out (32,256,56,56) f32.

Strategy
- Shard the batch dim across the 8 cores (4 images each); replicate W and b.
- Host-side prep: zero-pad x to 58x58 (so every shifted read in the kernel is
  a plain strided AP, no edge cases), and pre-transpose W into the stationary
  [ci, (tap, co)] layout the tensor engine wants.
- Per core: keep the whole padded shard (6.9 MB) + weights in SBUF. For each
  (image, 8-row block, co-half): accumulate 9 shifted matmuls (one per tap)
  into one PSUM bank, contraction dim = C_in = 128 (full partition width),
  moving dim N = 8*56 = 448. Weights/activations are loaded as float32r via
  casting SWDGE DMAs (full 1 cycle/row PE rate at N>=256, vs 4 cycles/row for
  plain fp32). ScalarE fuses the bias-add with the PSUM->SBUF copy, and the
  result DMAs out from ScalarE's HWDGE ring.
"""

import numpy as np

import bass_rust as _br
import concourse.bass as bass
import concourse.mybir as mybir
import concourse.tile as tile
from concourse.bass_utils import run_bass_kernel_spmd

N_CORES = 8
B, C_IN, H, W_ = 32, 128, 56, 56
C_OUT = 256
B_LOC = B // N_CORES          # 4 images per core
HP, WP = H + 2, W_ + 2        # padded 58x58
IMG_PAD = HP * WP             # 3364
ROWS_PER_BLK = 8              # 8 rows * 56 cols = 448 = moving dim (<=512 f32)
N_BLK = H // ROWS_PER_BLK     # 7
N_MOV = ROWS_PER_BLK * W_     # 448
N_TAPS = 9
CO_TILES = C_OUT // 128       # 2

_F32 = mybir.dt.float32
_F32R = mybir.dt.float32r


def _split_multi_waits(nc):
    """This walrus build accepts at most ONE sync-wait per instruction.

    Tile can emit several (e.g. a matmul waiting on two input DMAs, or the
    exit drain waiting on every outstanding semaphore). Hoist the extras onto
    injected same-engine NOPs immediately ahead of the offender — sequencers
    execute their stream in order, so the waits still all happen before it.
    """
    for bb in nc.m.functions[0].blocks:
        il = bb.instructions
        i = 0
        while i < len(il):
            inst = il[i]
            si = inst.sync_info
            w = list(si.on_wait) if (si and si.on_wait) else []
            if len(w) > 1:
                si.on_wait = w[-1:]
                for wi in w[:-1]:
                    nop = mybir.InstNoOp(
                        name=nc.get_next_instruction_name(), ins=[], outs=[]
                    )
                    nop.engine = inst.engine
                    nop.sync_info = _br.SyncInfo(on_wait=[wi], on_update=[])
                    nc.register_instruction(nop)
                    il.insert(i, nop)
                    i += 1
            i += 1


def _build_program():
    nc = bass.Bass("TRN2", target_bir_lowering=False, debug=False,
                   num_devices=N_CORES)
    xp = nc.dram_tensor("xp", [B_LOC, C_IN, IMG_PAD], _F32,
                        kind="ExternalInput").ap()
    wt = nc.dram_tensor("wt", [C_IN, N_TAPS * C_OUT], _F32,
                        kind="ExternalInput").ap()
    bt = nc.dram_tensor("bt", [128, CO_TILES], _F32, kind="ExternalInput").ap()
    out = nc.dram_tensor("out", [B_LOC, C_OUT, H, W_], _F32,
                         kind="ExternalOutput").ap()

    with tile.TileContext(nc) as tc:
        with (
            tc.tile_pool(name="xpool", bufs=1) as xpool,
            tc.tile_pool(name="wpool", bufs=1) as wpool,
            tc.tile_pool(name="opool", bufs=4) as opool,
            tc.tile_pool(name="ppool", bufs=4, space="PSUM") as ppool,
        ):
            # f32 -> f32r casting loads must go through SWDGE (gpsimd)
            w_sb = wpool.tile([C_IN, N_TAPS * C_OUT], _F32R, tag="w")
            nc.gpsimd.dma_start(w_sb[:], wt[:])
            b_sb = wpool.tile([128, CO_TILES], _F32, tag="b")
            nc.sync.dma_start(b_sb[:], bt[:])

            # Split each image's load so the first matmuls start after ~1/4
            # of an image instead of waiting for the whole 1.7 MB transfer.
            x_sb = []
            for n in range(B_LOC):
                t_ = xpool.tile([C_IN, IMG_PAD], _F32R, tag=f"x{n}")
                step = IMG_PAD // 4
                for s in range(4):
                    hi = IMG_PAD if s == 3 else (s + 1) * step
                    nc.gpsimd.dma_start(t_[:, s * step:hi],
                                        xp[n][:, s * step:hi])
                x_sb.append(t_)

            for n in range(B_LOC):
                xv = x_sb[n][:].rearrange("p (h w) -> p h w", h=HP, w=WP)
                for j in range(N_BLK):
                    for t in range(CO_TILES):
                        ps = ppool.tile([128, N_MOV], _F32, tag="ps")
                        for k in range(N_TAPS):
                            kh, kw = divmod(k, 3)
                            r0 = j * ROWS_PER_BLK + kh
                            rhs = xv[:, r0:r0 + ROWS_PER_BLK, kw:kw + W_]
                            lhsT = w_sb[:, k * C_OUT + t * 128:
                                        k * C_OUT + t * 128 + 128]
                            nc.tensor.matmul(
                                ps[:], lhsT, rhs,
                                start=(k == 0),
                                stop=(k == N_TAPS - 1),
                            )
                        o_sb = opool.tile([128, N_MOV], _F32, tag="o")
                        nc.scalar.activation(
                            o_sb[:], ps[:],
                            mybir.ActivationFunctionType.Identity,
                            bias=b_sb[:, t:t + 1],
                        )
                        nc.scalar.dma_start(
                            out[n, bass.ts(t, 128), bass.ts(j, ROWS_PER_BLK), :],
                            o_sb[:],
                        )

    _split_multi_waits(nc)
    return nc


_CACHED_NC = None


def _get_program():
    global _CACHED_NC
    if _CACHED_NC is None:
        _CACHED_NC = _build_program()
    return _CACHED_NC


def _prep_inputs(x, W, b):
    xp_all = np.pad(x, ((0, 0), (0, 0), (1, 1), (1, 1)))
    wt = np.ascontiguousarray(
        W.reshape(C_OUT, C_IN, N_TAPS).transpose(1, 2, 0).reshape(C_IN, -1)
    )
    bt = np.ascontiguousarray(b.reshape(CO_TILES, 128).T)
    in_maps = []
    for i in range(N_CORES):
        shard = np.ascontiguousarray(
            xp_all[i * B_LOC:(i + 1) * B_LOC].reshape(B_LOC, C_IN, IMG_PAD)
        )
        in_maps.append({"xp": shard, "wt": wt, "bt": bt})
    return in_maps


def kernel(x, W, b):
    x = np.asarray(x, dtype=np.float32)
    W = np.asarray(W, dtype=np.float32)
    b = np.asarray(b, dtype=np.float32)
    nc = _get_program()
    in_maps = _prep_inputs(x, W, b)
    res = run_bass_kernel_spmd(nc, in_maps, list(range(N_CORES)), trace=False)
    return np.concatenate([res.results[i]["out"] for i in range(N_CORES)], axis=0)


# revision 4
# speedup vs baseline: 1.0134x; 1.0134x over previous
"""Data-parallel 3x3 conv (implicit GEMM) for Trainium2, 8 NeuronCores.

Problem: x (32,128,56,56) f32, W (256,1152) f32 [C_out, C_in*KH*KW, taps in
(ci,kh,kw) order], b (256,), stride 1, pad 1 -> out (32,256,56,56) f32.

Strategy
- Shard the batch dim across the 8 cores (4 images each); replicate W and b.
- Host-side prep: zero-pad x to 58x58 (so every shifted read in the kernel is
  a plain strided AP, no edge cases), and pre-transpose W into the stationary
  [ci, (tap, co)] layout the tensor engine wants.
- Per core: keep the whole padded shard (6.9 MB) + weights in SBUF. For each
  (image, 8-row block, co-half): accumulate 9 shifted matmuls (one per tap)
  into one PSUM bank, contraction dim = C_in = 128 (full partition width),
  moving dim N = 8*56 = 448. Weights/activations are loaded as float32r via
  casting SWDGE DMAs (full 1 cycle/row PE rate at N>=256, vs 4 cycles/row for
  plain fp32). ScalarE fuses the bias-add with the PSUM->SBUF copy, and the
  result DMAs out from ScalarE's HWDGE ring.
"""

import numpy as np

import bass_rust as _br
import concourse.bass as bass
import concourse.mybir as mybir
import concourse.tile as tile
from concourse.bass_utils import run_bass_kernel_spmd

N_CORES = 8
B, C_IN, H, W_ = 32, 128, 56, 56
C_OUT = 256
B_LOC = B // N_CORES          # 4 images per core
HP, WP = H + 2, W_ + 2        # padded 58x58
IMG_PAD = HP * WP             # 3364
ROWS_PER_BLK = 8              # 8 rows * 56 cols = 448 = moving dim (<=512 f32)
N_BLK = H // ROWS_PER_BLK     # 7
N_MOV = ROWS_PER_BLK * W_     # 448
N_TAPS = 9
CO_TILES = C_OUT // 128       # 2

_F32 = mybir.dt.float32
_F32R = mybir.dt.float32r


def _split_multi_waits(nc):
    """This walrus build accepts at most ONE sync-wait per instruction.

    Tile can emit several (e.g. a matmul waiting on two input DMAs, or the
    exit drain waiting on every outstanding semaphore). Hoist the extras onto
    injected same-engine NOPs immediately ahead of the offender — sequencers
    execute their stream in order, so the waits still all happen before it.
    """
    for bb in nc.m.functions[0].blocks:
        il = bb.instructions
        i = 0
        while i < len(il):
            inst = il[i]
            si = inst.sync_info
            w = list(si.on_wait) if (si and si.on_wait) else []
            if len(w) > 1:
                si.on_wait = w[-1:]
                for wi in w[:-1]:
                    nop = mybir.InstNoOp(
                        name=nc.get_next_instruction_name(), ins=[], outs=[]
                    )
                    nop.engine = inst.engine
                    nop.sync_info = _br.SyncInfo(on_wait=[wi], on_update=[])
                    nc.register_instruction(nop)
                    il.insert(i, nop)
                    i += 1
            i += 1


def _build_program():
    nc = bass.Bass("TRN2", target_bir_lowering=False, debug=False,
                   num_devices=N_CORES)
    xp = nc.dram_tensor("xp", [B_LOC, C_IN, IMG_PAD], _F32,
                        kind="ExternalInput").ap()
    wt = nc.dram_tensor("wt", [C_IN, N_TAPS * C_OUT], _F32,
                        kind="ExternalInput").ap()
    bt = nc.dram_tensor("bt", [128, CO_TILES], _F32, kind="ExternalInput").ap()
    out = nc.dram_tensor("out", [B_LOC, C_OUT, H, W_], _F32,
                         kind="ExternalOutput").ap()

    with tile.TileContext(nc) as tc:
        with (
            tc.tile_pool(name="xpool", bufs=1) as xpool,
            tc.tile_pool(name="wpool", bufs=1) as wpool,
            tc.tile_pool(name="opool", bufs=4) as opool,
            tc.tile_pool(name="ppool", bufs=4, space="PSUM") as ppool,
        ):
            # f32 -> f32r casting loads must go through SWDGE (gpsimd)
            w_sb = wpool.tile([C_IN, N_TAPS * C_OUT], _F32R, tag="w")
            nc.gpsimd.dma_start(w_sb[:], wt[:])
            b_sb = wpool.tile([128, CO_TILES], _F32, tag="b")
            nc.sync.dma_start(b_sb[:], bt[:])

            # Split each image's load so the first matmuls start after ~1/4
            # of an image instead of waiting for the whole 1.7 MB transfer.
            x_sb = []
            for n in range(B_LOC):
                t_ = xpool.tile([C_IN, IMG_PAD], _F32R, tag=f"x{n}")
                n_split = 8
                step = IMG_PAD // n_split
                for s in range(n_split):
                    hi = IMG_PAD if s == n_split - 1 else (s + 1) * step
                    nc.gpsimd.dma_start(t_[:, s * step:hi],
                                        xp[n][:, s * step:hi])
                x_sb.append(t_)

            for n in range(B_LOC):
                xv = x_sb[n][:].rearrange("p (h w) -> p h w", h=HP, w=WP)
                for j in range(N_BLK):
                    for t in range(CO_TILES):
                        ps = ppool.tile([128, N_MOV], _F32, tag="ps")
                        for k in range(N_TAPS):
                            kh, kw = divmod(k, 3)
                            r0 = j * ROWS_PER_BLK + kh
                            rhs = xv[:, r0:r0 + ROWS_PER_BLK, kw:kw + W_]
                            lhsT = w_sb[:, k * C_OUT + t * 128:
                                        k * C_OUT + t * 128 + 128]
                            nc.tensor.matmul(
                                ps[:], lhsT, rhs,
                                start=(k == 0),
                                stop=(k == N_TAPS - 1),
                            )
                        o_sb = opool.tile([128, N_MOV], _F32, tag="o")
                        nc.scalar.activation(
                            o_sb[:], ps[:],
                            mybir.ActivationFunctionType.Identity,
                            bias=b_sb[:, t:t + 1],
                        )
                        nc.scalar.dma_start(
                            out[n, bass.ts(t, 128), bass.ts(j, ROWS_PER_BLK), :],
                            o_sb[:],
                        )

    _split_multi_waits(nc)
    return nc


_CACHED_NC = None


def _get_program():
    global _CACHED_NC
    if _CACHED_NC is None:
        _CACHED_NC = _build_program()
    return _CACHED_NC


def _prep_inputs(x, W, b):
    xp_all = np.pad(x, ((0, 0), (0, 0), (1, 1), (1, 1)))
    wt = np.ascontiguousarray(
        W.reshape(C_OUT, C_IN, N_TAPS).transpose(1, 2, 0).reshape(C_IN, -1)
    )
    bt = np.ascontiguousarray(b.reshape(CO_TILES, 128).T)
    in_maps = []
    for i in range(N_CORES):
        shard = np.ascontiguousarray(
            xp_all[i * B_LOC:(i + 1) * B_LOC].reshape(B_LOC, C_IN, IMG_PAD)
        )
        in_maps.append({"xp": shard, "wt": wt, "bt": bt})
    return in_maps


def kernel(x, W, b):
    x = np.asarray(x, dtype=np.float32)
    W = np.asarray(W, dtype=np.float32)
    b = np.asarray(b, dtype=np.float32)
    nc = _get_program()
    in_maps = _prep_inputs(x, W, b)
    res = run_bass_kernel_spmd(nc, in_maps, list(range(N_CORES)), trace=False)
    return np.concatenate([res.results[i]["out"] for i in range(N_CORES)], axis=0)


# revision 5
# speedup vs baseline: 61.8558x; 61.0359x over previous
"""Data-parallel 3x3 conv (implicit GEMM) for Trainium2, 8 NeuronCores.

Problem: x (32,128,56,56) f32, W (256,1152) f32 [C_out, C_in*KH*KW, taps in
(ci,kh,kw) order], b (256,), stride 1, pad 1 -> out (32,256,56,56) f32.

Strategy
- Shard the batch dim across the 8 cores (4 images each); replicate W and b.
- Host-side prep: zero-pad x to 58x58 (so every shifted read in the kernel is
  a plain strided AP, no edge cases), and pre-transpose W into the stationary
  [ci, (tap, co)] layout the tensor engine wants.
- Per core: keep the whole padded shard (6.9 MB) + weights in SBUF. For each
  (image, 8-row block, co-half): accumulate 9 shifted matmuls (one per tap)
  into one PSUM bank, contraction dim = C_in = 128 (full partition width),
  moving dim N = 8*56 = 448. Weights/activations are loaded as float32r via
  casting SWDGE DMAs (full 1 cycle/row PE rate at N>=256, vs 4 cycles/row for
  plain fp32). ScalarE fuses the bias-add with the PSUM->SBUF copy, and the
  result DMAs out from ScalarE's HWDGE ring.
"""

import numpy as np

import bass_rust as _br
import concourse.bass as bass
import concourse.mybir as mybir
import concourse.tile as tile
from concourse.bass_utils import run_bass_kernel_spmd

N_CORES = 8
B, C_IN, H, W_ = 32, 128, 56, 56
C_OUT = 256
B_LOC = B // N_CORES          # 4 images per core
HP, WP = H + 2, W_ + 2        # padded 58x58
IMG_PAD = HP * WP             # 3364
ROWS_PER_BLK = 8              # 8 rows * 56 cols = 448 = moving dim (<=512 f32)
N_BLK = H // ROWS_PER_BLK     # 7
N_MOV = ROWS_PER_BLK * W_     # 448
N_TAPS = 9
CO_TILES = C_OUT // 128       # 2

_F32 = mybir.dt.float32
_F32R = mybir.dt.float32r


def _split_multi_waits(nc):
    """This walrus build accepts at most ONE sync-wait per instruction.

    Tile can emit several (e.g. a matmul waiting on two input DMAs, or the
    exit drain waiting on every outstanding semaphore). Hoist the extras onto
    injected same-engine NOPs immediately ahead of the offender — sequencers
    execute their stream in order, so the waits still all happen before it.
    """
    for bb in nc.m.functions[0].blocks:
        il = bb.instructions
        i = 0
        while i < len(il):
            inst = il[i]
            si = inst.sync_info
            w = list(si.on_wait) if (si and si.on_wait) else []
            if len(w) > 1:
                si.on_wait = w[-1:]
                for wi in w[:-1]:
                    nop = mybir.InstNoOp(
                        name=nc.get_next_instruction_name(), ins=[], outs=[]
                    )
                    nop.engine = inst.engine
                    nop.sync_info = _br.SyncInfo(on_wait=[wi], on_update=[])
                    nc.register_instruction(nop)
                    il.insert(i, nop)
                    i += 1
            i += 1


def _build_program():
    nc = bass.Bass("TRN2", target_bir_lowering=False, debug=False,
                   num_devices=N_CORES)
    xp = nc.dram_tensor("xp", [B_LOC, C_IN, IMG_PAD], _F32,
                        kind="ExternalInput").ap()
    wt = nc.dram_tensor("wt", [C_IN, N_TAPS * C_OUT], _F32,
                        kind="ExternalInput").ap()
    bt = nc.dram_tensor("bt", [128, CO_TILES], _F32, kind="ExternalInput").ap()
    out = nc.dram_tensor("out", [B_LOC, C_OUT, H, W_], _F32,
                         kind="ExternalOutput").ap()

    with tile.TileContext(nc) as tc:
        with (
            tc.tile_pool(name="xpool", bufs=1) as xpool,
            tc.tile_pool(name="wpool", bufs=1) as wpool,
            tc.tile_pool(name="opool", bufs=4) as opool,
            tc.tile_pool(name="ppool", bufs=4, space="PSUM") as ppool,
        ):
            # f32 -> f32r casting loads must go through SWDGE (gpsimd).
            # Split W in two and give image 0 a small leading chunk so the
            # first matmul group isn't serialized behind whole-tensor loads
            # (cost model: 113.8 -> 111.4 us).
            w_sb = wpool.tile([C_IN, N_TAPS * C_OUT], _F32R, tag="w")
            wcols = N_TAPS * C_OUT
            nc.gpsimd.dma_start(w_sb[:, :wcols // 2], wt[:, :wcols // 2])
            nc.gpsimd.dma_start(w_sb[:, wcols // 2:], wt[:, wcols // 2:])
            b_sb = wpool.tile([128, CO_TILES], _F32, tag="b")
            nc.sync.dma_start(b_sb[:], bt[:])

            x_sb = []
            for n in range(B_LOC):
                t_ = xpool.tile([C_IN, IMG_PAD], _F32R, tag=f"x{n}")
                if n == 0:
                    bounds = [0, IMG_PAD // 16, IMG_PAD // 4, IMG_PAD // 2,
                              3 * IMG_PAD // 4, IMG_PAD]
                else:
                    bounds = [0, IMG_PAD // 4, IMG_PAD // 2,
                              3 * IMG_PAD // 4, IMG_PAD]
                for lo, hi in zip(bounds[:-1], bounds[1:]):
                    nc.gpsimd.dma_start(t_[:, lo:hi], xp[n][:, lo:hi])
                x_sb.append(t_)

            for n in range(B_LOC):
                xv = x_sb[n][:].rearrange("p (h w) -> p h w", h=HP, w=WP)
                for j in range(N_BLK):
                    for t in range(CO_TILES):
                        ps = ppool.tile([128, N_MOV], _F32, tag="ps")
                        for k in range(N_TAPS):
                            kh, kw = divmod(k, 3)
                            r0 = j * ROWS_PER_BLK + kh
                            rhs = xv[:, r0:r0 + ROWS_PER_BLK, kw:kw + W_]
                            lhsT = w_sb[:, k * C_OUT + t * 128:
                                        k * C_OUT + t * 128 + 128]
                            nc.tensor.matmul(
                                ps[:], lhsT, rhs,
                                start=(k == 0),
                                stop=(k == N_TAPS - 1),
                            )
                        o_sb = opool.tile([128, N_MOV], _F32, tag="o")
                        nc.scalar.activation(
                            o_sb[:], ps[:],
                            mybir.ActivationFunctionType.Identity,
                            bias=b_sb[:, t:t + 1],
                        )
                        nc.scalar.dma_start(
                            out[n, bass.ts(t, 128), bass.ts(j, ROWS_PER_BLK), :],
                            o_sb[:],
                        )

    _split_multi_waits(nc)
    return nc


_CACHED_NC = None


def _get_program():
    global _CACHED_NC
    if _CACHED_NC is None:
        _CACHED_NC = _build_program()
    return _CACHED_NC


def _prep_inputs(x, W, b):
    xp_all = np.pad(x, ((0, 0), (0, 0), (1, 1), (1, 1)))
    wt = np.ascontiguousarray(
        W.reshape(C_OUT, C_IN, N_TAPS).transpose(1, 2, 0).reshape(C_IN, -1)
    )
    bt = np.ascontiguousarray(b.reshape(CO_TILES, 128).T)
    in_maps = []
    for i in range(N_CORES):
        shard = np.ascontiguousarray(
            xp_all[i * B_LOC:(i + 1) * B_LOC].reshape(B_LOC, C_IN, IMG_PAD)
        )
        in_maps.append({"xp": shard, "wt": wt, "bt": bt})
    return in_maps


def kernel(x, W, b):
    x = np.asarray(x, dtype=np.float32)
    W = np.asarray(W, dtype=np.float32)
    b = np.asarray(b, dtype=np.float32)
    nc = _get_program()
    in_maps = _prep_inputs(x, W, b)
    res = run_bass_kernel_spmd(nc, in_maps, list(range(N_CORES)), trace=False)
    return np.concatenate([res.results[i]["out"] for i in range(N_CORES)], axis=0)
